# revision 16
# baseline (speedup 1.0000x reference)
"""Trainium2 Bass kernel for LlamaSwiftKV-style attention.

Full (unsharded) inputs in, full output out. Internally tensor-parallel
over 8 NeuronCores: core c owns kv-head c and q-heads 4c..4c+3, i.e. a
512-wide slice of the q/o projection feature dim. Each core computes a
partial output projection [B*Q, HID]; the partials are summed on host.

The kernel is HBM-DMA-bound, so the big lever is precision: every
HBM-resident operand (x, q_w, K, V, o_w) is downcast to fp16 on the
host (all values are O(1); fp16 keeps ~5e-4 relative accuracy, well
inside the 2e-2 gate) which halves the DMA stream to ~76us. All matmul
accumulation stays in fp32 PSUM; softmax statistics stay fp32.

Structure (per core):
  - q-proj fp16: psum [64, 512] accumulated over 32 k-chunks, consuming
    the q_w stream as it lands
  - RoPE on the free axis in fp32; 1/sqrt(D) folded into the host
    cos/sin tables; per-head PE transpose -> qT [d, (g,b,q)] in fp16
  - scores^T per (batch, 16-chunk group): kv on partitions, exp is one
    ACT op per group -> E [128, 32, 32] fp16; causal mask only affects
    the last kv chunk (one tiny DVE bias-add)
  - denominator (ones^T E) + reciprocal + rank-1 broadcast right after
    exp so only P@V trails the V DMA
  - P@V accumulated over 32 chunks -> outT psum [d, 32]; normalize into
    attnT fp16
  - o-proj transposed layout: out[hid-chunk(p), 8 tokens-col grp] via
    lhsT = o_w 128x128 chunks, rhs = attnT -> 128 matmuls of 64 rows
    (fp16 streams 1 row/cycle at any width); stores are 2KB-contiguous
    in a [128, 32, 64] DRAM layout the host unswizzles
  - o_w streams on the otherwise-idle Pool queue; the last quarter is
    held until batch 7's V so tail compute overlaps the final transfer
"""

import sys

for _p in ("/opt/trn_rl_repo", "/root/.axon_site/_ro/trn_rl_repo"):
    if _p not in sys.path:
        sys.path.append(_p)

import numpy as np

B, Q, HID = 8, 8, 4096
H, KVH, D = 32, 8, 128
KV = 4096
ROPE_THETA = 10000.0
NCORES = 8
G = H // KVH            # 4 q-heads per kv-head (= per core)
FEAT = G * D            # 512 feature slice per core
T = B * Q               # 64 tokens
NCHUNK = KV // 128      # 32 kv chunks
HALF = D // 2
GQ = G * Q              # 32 score columns per batch

_CACHE = {}


def _build_program():
    import concourse.bass as bass
    import concourse.tile as tile
    from concourse import bacc, mybir
    from concourse.masks import make_identity
    from concourse.tile_rust import add_dep_helper
    from contextlib import ExitStack

    f32 = mybir.dt.float32
    f16 = mybir.dt.float16
    nc = bacc.Bacc("TRN2", target_bir_lowering=False, debug=False)

    xT_d = nc.dram_tensor("xt", [128, HID // 128, T], f16, kind="ExternalInput")
    qwT_d = nc.dram_tensor("qwt", [HID, FEAT], f16, kind="ExternalInput")
    owT_d = nc.dram_tensor("owt", [FEAT, HID], f16, kind="ExternalInput")
    kT_d = nc.dram_tensor("kt", [B, D, KV], f16, kind="ExternalInput")
    # v pre-swizzled on host: [B, 128(p), 32(chunk), 128(d)]
    v_d = nc.dram_tensor("v", [B, 128, NCHUNK, D], f16, kind="ExternalInput")
    # mask bias for the last kv chunk only (causal tail): [128(p), B, 32(g*q)]
    mb_d = nc.dram_tensor("mb", [128, B, GQ], f32, kind="ExternalInput")
    ones_d = nc.dram_tensor("ones", [128, 1], f16, kind="ExternalInput")
    cosb_d = nc.dram_tensor("cosb", [T, FEAT], f32, kind="ExternalInput")
    sinb_d = nc.dram_tensor("sinb", [T, FEAT], f32, kind="ExternalInput")
    # output transposed+swizzled [128(p), 32(hid chunk), 64(t)] fp16 (the
    # per-core partial is fp16; the host sums in fp32); host unswizzles
    # hid = c*128 + p
    out_d = nc.dram_tensor("out", [128, HID // 128, T], f16, kind="ExternalOutput")

    with tile.TileContext(nc) as tc, ExitStack() as ctx:
        const = ctx.enter_context(tc.tile_pool(name="const", bufs=1))
        qw_pool = ctx.enter_context(tc.tile_pool(name="qw", bufs=4))
        kt_pool = ctx.enter_context(tc.tile_pool(name="kt", bufs=2))
        v_pool = ctx.enter_context(tc.tile_pool(name="v", bufs=2))
        e_pool = ctx.enter_context(tc.tile_pool(name="e", bufs=2))
        small = ctx.enter_context(tc.tile_pool(name="small", bufs=4))
        rope_pool = ctx.enter_context(tc.tile_pool(name="rope", bufs=1))
        out_pool = ctx.enter_context(tc.tile_pool(name="outp", bufs=4))
        ps_s = ctx.enter_context(tc.tile_pool(name="ps_s", bufs=2, space="PSUM"))
        ps_o = ctx.enter_context(tc.tile_pool(name="ps_o", bufs=2, space="PSUM"))
        ps_d = ctx.enter_context(tc.tile_pool(name="ps_d", bufs=1, space="PSUM"))
        ps_b = ctx.enter_context(tc.tile_pool(name="ps_b", bufs=2, space="PSUM"))

        Exp = mybir.ActivationFunctionType.Exp
        Copy = mybir.ActivationFunctionType.Copy

        # x^T staged as [128, 32(chunk), 64] (host-swizzled, contiguous).
        # Issued first so the small DMAs' descriptor generation hides
        # under its transfer.
        xt = const.tile([128, HID // 128, T], f16)
        nc.sync.dma_start(out=xt, in_=xT_d.ap())
        ones_kv = const.tile([128, 1], f16)
        nc.sync.dma_start(out=ones_kv, in_=ones_d.ap())
        ones_bc = const.tile([1, 128], f32)
        nc.vector.memset(ones_bc, 1.0)
        ident = const.tile([T, T], f32)
        make_identity(nc, ident)
        cosb = const.tile([T, FEAT], f32)
        nc.sync.dma_start(out=cosb, in_=cosb_d.ap())
        sinb = const.tile([T, FEAT], f32)
        nc.sync.dma_start(out=sinb, in_=sinb_d.ap())
        mb31 = const.tile([128, B, GQ], f32)
        nc.sync.dma_start(out=mb31, in_=mb_d.ap())

        # ---- q projection: psum [64, 512] accumulated over 32 k-chunks
        q_ps = ps_b.tile([T, FEAT], f32, tag="misc")
        nkc = HID // 128
        QCH = 4
        qw_dmas = []
        for cgrp in range(nkc // QCH):
            qw_t = qw_pool.tile([128, QCH, FEAT], f16)
            qw_dmas.append(nc.gpsimd.dma_start(
                out=qw_t,
                in_=qwT_d.ap()
                .rearrange("(c p) f -> p c f", p=128)[
                    :, QCH * cgrp : QCH * (cgrp + 1), :
                ],
            ))
            for i in range(QCH):
                c = QCH * cgrp + i
                nc.tensor.matmul(
                    q_ps, xt[:, c, :], qw_t[:, i, :],
                    start=(c == 0), stop=(c == nkc - 1),
                )

        # ---- RoPE on the free axis (feat = g*128 + d); 1/sqrt(D) folded
        # into the host cos/sin tables
        qv = q_ps.rearrange("t (g h d) -> t g h d", g=G, h=2)
        rot = rope_pool.tile([T, G, 2, HALF], f32)
        nc.vector.tensor_copy(rot[:, :, 0, :], qv[:, :, 1, :])
        nc.vector.tensor_copy(rot[:, :, 1, :], qv[:, :, 0, :])
        q_rope = rope_pool.tile([T, FEAT], f32)
        nc.vector.tensor_mul(q_rope, q_ps, cosb)
        rot_f = rot.rearrange("t g h d -> t (g h d)")
        nc.vector.tensor_mul(rot_f, rot_f, sinb)
        nc.vector.tensor_add(q_rope, q_rope, rot_f)

        # ---- transpose each head -> qT [128(d), G, 64(b,q)] fp16
        qT = const.tile([128, G, T], f16)
        for g in range(G):
            tp = ps_b.tile([128, T], f32, tag="misc")
            nc.tensor.transpose(tp, q_rope[:, g * 128 : (g + 1) * 128], ident)
            nc.vector.tensor_copy(qT[:, g, :], tp)

        # attention output (transposed, normalized) [128(d), G, 64(b,q)]
        attnT = const.tile([128, G, T], f16)

        ow_t = const.tile([128, G, HID], f16)
        ow_dmas = {}

        def issue_ow(qi, pace_dma):
            # o_w streams on the (otherwise idle after q_w) Pool queue;
            # quarters 0-2 pace into the k/v stream, quarter 3 is held
            # until batch 7's V so tail compute overlaps its transfer
            owq = HID // 4
            dma = nc.gpsimd.dma_start(
                out=ow_t[:, :, qi * owq : (qi + 1) * owq],
                in_=owT_d.ap().rearrange("(g p) n -> p g n", p=128)[
                    :, :, qi * owq : (qi + 1) * owq
                ],
            )
            add_dep_helper(
                dma.ins, pace_dma.ins, sync=True,
                reason="pace ow quarter into the k/v stream",
            )
            ow_dmas[qi] = dma

        # ---- per-batch attention
        for b in range(B):
            kt_t = kt_pool.tile([128, KV], f16)
            kt_dma0 = nc.sync.dma_start(
                out=kt_t[:, : KV // 2], in_=kT_d.ap()[b][:, : KV // 2]
            )
            kt_dma1 = nc.sync.dma_start(
                out=kt_t[:, KV // 2 :], in_=kT_d.ap()[b][:, KV // 2 :]
            )
            v_t = v_pool.tile([128, NCHUNK, D], f16)
            v_dmas = []
            nvd = 4 if b == B - 1 else 1
            vch = NCHUNK // nvd
            for vi in range(nvd):
                v_dmas.append(nc.sync.dma_start(
                    out=v_t[:, vi * vch : (vi + 1) * vch, :],
                    in_=v_d.ap()[b][:, vi * vch : (vi + 1) * vch, :],
                ))
            if b == 0:
                # keep the q-proj weight stream ahead of batch prefetch;
                # gate on the 3rd-last qw DMA so this DMA's descriptor
                # generation overlaps the last qw transfers
                for d_inst in (kt_dma0, kt_dma1, *v_dmas):
                    add_dep_helper(
                        d_inst.ins,
                        qw_dmas[-3].ins,
                        sync=True,
                        reason="batch prefetch after q-proj weights",
                    )
            if b == 1:
                issue_ow(0, kt_dma1)
            elif b == 3:
                issue_ow(1, kt_dma1)
            elif b == 5:
                issue_ow(2, kt_dma1)
            elif b == B - 1:
                # pace on batch 7's second kt half: the ~2.2us SWDGE
                # generation + sem latency hides under the 4 v quarters,
                # so the transfer starts right as the last v quarter ends
                issue_ow(3, kt_dma1)

            # scores^T per 16-chunk group; exp is one ACT op per group
            e_t = e_pool.tile([128, NCHUNK, GQ], f16)
            for cg in range(2):
                s_ps = ps_s.tile([128, 16 * GQ], f32)
                for cc in range(16):
                    c = cg * 16 + cc
                    nc.tensor.matmul(
                        s_ps[:, cc * GQ : (cc + 1) * GQ],
                        kt_t[:, c * 128 : (c + 1) * 128],
                        qT[:, :, b * Q : (b + 1) * Q],
                        start=True,
                        stop=True,
                    )
                if cg == 1:
                    # causal mask only affects the last kv chunk
                    nc.vector.tensor_add(
                        s_ps[:, 15 * GQ :], s_ps[:, 15 * GQ :], mb31[:, b, :]
                    )
                nc.scalar.activation(
                    e_t[:, cg * 16 : (cg + 1) * 16, :].rearrange(
                        "p c j -> p (c j)"
                    ),
                    s_ps,
                    Exp,
                )

            # denominator right after exp (depends only on K): ones^T @ E,
            # chunk-halves folded in psum, then reduce + reciprocal + bcast
            d_ps = ps_d.tile([1, 16 * GQ], f32)
            nc.tensor.matmul(
                d_ps,
                ones_kv,
                e_t[:, 0:16, :].rearrange("p c j -> p (c j)"),
                start=True,
                stop=False,
            )
            nc.tensor.matmul(
                d_ps,
                ones_kv,
                e_t[:, 16:32, :].rearrange("p c j -> p (c j)"),
                start=False,
                stop=True,
            )
            den = small.tile([1, GQ], f32)
            nc.vector.reduce_sum(
                den,
                d_ps.rearrange("p (c j) -> p j c", c=16),
                axis=mybir.AxisListType.X,
            )
            rec = small.tile([1, GQ], f32)
            nc.vector.reciprocal(rec, den)
            bc_ps = ps_d.tile([128, GQ], f32, tag="bc")
            nc.tensor.matmul(bc_ps, ones_bc, rec, start=True, stop=True)

            # P @ V -> outT psum [d=128, 32]
            o_ps = ps_o.tile([128, GQ], f32, tag="o")
            for c in range(NCHUNK):
                nc.tensor.matmul(
                    o_ps,
                    v_t[:, c, :],
                    e_t[:, c, :],
                    start=(c == 0),
                    stop=(c == NCHUNK - 1),
                )

            o_sb = small.tile([128, GQ], f32)
            nc.scalar.activation(o_sb, o_ps, Copy)
            nc.vector.tensor_mul(
                attnT[:, :, b * Q : (b + 1) * Q],
                o_sb.rearrange("p (g q) -> p g q", g=G),
                bc_ps.rearrange("p (g q) -> p g q", g=G),
            )

        # ---- o-proj, transposed: outT[hid(p), t] via o_w 128x128 chunks
        # stationary, attnT moving -> 64-row fp16 matmuls (1 row/cycle).
        # 8 fine groups so copies+stores pipeline tightly behind the
        # matmul stream in the tail.
        OHC = 4  # hid 128-chunks per group
        for hg in range(HID // (128 * OHC)):
            op_ps = ps_b.tile([128, OHC, T], f32, tag="misc", name=f"op_{hg}")
            for i in range(OHC):
                hc = hg * OHC + i
                for g in range(G):
                    nc.tensor.matmul(
                        op_ps[:, i, :],
                        ow_t[:, g, hc * 128 : (hc + 1) * 128],
                        attnT[:, g, :],
                        start=(g == 0),
                        stop=(g == G - 1),
                    )
            ot = out_pool.tile([128, OHC, T], f16, tag="ot")
            nc.scalar.activation(ot, op_ps, Copy)
            nc.sync.dma_start(
                out=out_d.ap()[:, hg * OHC : (hg + 1) * OHC, :],
                in_=ot,
            )

    nc.compile()
    return nc


def _get_program():
    if "nc" not in _CACHE:
        _CACHE["nc"] = _build_program()
    return _CACHE["nc"]


def _host_prep(hidden_states, position_ids, key_cache, value_cache, attention_mask, q_w, o_w):
    """Build the per-core input maps (all host-side layout marshaling)."""
    x = np.asarray(hidden_states, np.float32).reshape(T, HID).astype(np.float16)
    xT = np.ascontiguousarray(x.T.reshape(HID // 128, 128, T).transpose(1, 0, 2))

    pos = np.asarray(position_ids)
    idx = int(np.argmax(pos[0].astype(np.int32)))
    pid = pos[:, idx].astype(np.float32)                      # [B]
    inv_freq = 1.0 / (ROPE_THETA ** (np.arange(0, HALF, dtype=np.float32) / HALF))
    ang = pid[:, None] * inv_freq[None, :]                    # [B, 64]
    emb = np.concatenate([ang, ang], axis=1)                  # [B, 128]
    scale = np.float32(1.0 / np.sqrt(D))                      # folded into RoPE
    cos_b = (np.cos(emb) * scale).astype(np.float32)
    sin_b = (np.sin(emb) * scale).astype(np.float32)
    sign = np.concatenate(
        [-np.ones(HALF, np.float32), np.ones(HALF, np.float32)]
    )
    sin_s = sin_b * sign[None, :]
    cosb = np.ascontiguousarray(np.tile(np.repeat(cos_b, Q, axis=0), (1, G)))
    sinb = np.ascontiguousarray(np.tile(np.repeat(sin_s, Q, axis=0), (1, G)))

    mask = np.asarray(attention_mask)[:, 0]                   # [B, Q, KV] bool
    mbias = np.where(mask, np.float32(-10000.0), np.float32(0.0))
    # only the last kv chunk is ever masked (causal tail); ship just that
    # chunk's bias, tiled over the G q-head groups: [128(p), B, G*Q]
    mb31 = mbias[:, :, KV - 128 :].transpose(0, 2, 1)         # [B, 128, Q]
    mb_host = np.ascontiguousarray(
        np.tile(mb31, (1, 1, G)).transpose(1, 0, 2)           # [128, B, G*Q]
    )

    kc = np.asarray(key_cache, np.float32).astype(np.float16)
    vc = np.asarray(value_cache, np.float32).astype(np.float16)
    qw = np.asarray(q_w, np.float32).astype(np.float16)
    ow = np.asarray(o_w, np.float32).astype(np.float16)

    in_maps = []
    for c in range(NCORES):
        kT = np.ascontiguousarray(kc[:, c].transpose(0, 2, 1))    # [B, D, KV]
        v_sw = np.ascontiguousarray(
            vc[:, c].reshape(B, NCHUNK, 128, D).transpose(0, 2, 1, 3)
        )                                                          # [B,128,32,128]
        qwT = np.ascontiguousarray(qw[c * FEAT : (c + 1) * FEAT, :].T)  # [HID, 512]
        owT = np.ascontiguousarray(ow[:, c * FEAT : (c + 1) * FEAT].T)  # [512, HID]
        in_maps.append(
            {
                "ones": np.ones((128, 1), np.float16),
                "xt": xT,
                "qwt": qwT,
                "owt": owT,
                "kt": kT,
                "v": v_sw,
                "mb": mb_host,
                "cosb": cosb,
                "sinb": sinb,
            }
        )
    return in_maps


def kernel(
    hidden_states,
    position_ids,
    key_cache,
    value_cache,
    attention_mask,
    q_w,
    o_w,
    _trace=False,
):
    from concourse.bass_utils import run_bass_kernel_spmd

    nc = _get_program()
    in_maps = _host_prep(
        hidden_states, position_ids, key_cache, value_cache, attention_mask, q_w, o_w
    )
    res = run_bass_kernel_spmd(nc, in_maps, list(range(NCORES)), trace=_trace)
    _CACHE["last_result"] = res
    out = np.zeros((T, HID), np.float32)
    for r in res.results:
        # r["out"] is fp16 [128(p), 32(c), 64(t)] with hid = c*128 + p
        out += r["out"].astype(np.float32).transpose(1, 0, 2).reshape(HID, T).T
    return out.reshape(B, Q, HID)


# revision 20
# speedup vs baseline: 1.0019x; 1.0019x over previous
"""Trainium2 Bass kernel for LlamaSwiftKV-style attention.

Full (unsharded) inputs in, full output out. Internally tensor-parallel
over 8 NeuronCores: core c owns kv-head c and q-heads 4c..4c+3, i.e. a
512-wide slice of the q/o projection feature dim. Each core computes a
partial output projection [B*Q, HID]; the partials are summed on host.

The kernel is HBM-DMA-bound, so the big lever is precision: every
HBM-resident operand (x, q_w, K, V, o_w) is downcast to fp16 on the
host (all values are O(1); fp16 keeps ~5e-4 relative accuracy, well
inside the 2e-2 gate) which halves the DMA stream to ~76us. All matmul
accumulation stays in fp32 PSUM; softmax statistics stay fp32.

Structure (per core):
  - q-proj fp16: psum [64, 512] accumulated over 32 k-chunks, consuming
    the q_w stream as it lands
  - RoPE on the free axis in fp32; 1/sqrt(D) folded into the host
    cos/sin tables; per-head PE transpose -> qT [d, (g,b,q)] in fp16
  - scores^T per (batch, 16-chunk group): kv on partitions, exp is one
    ACT op per group -> E [128, 32, 32] fp16; causal mask only affects
    the last kv chunk (one tiny DVE bias-add)
  - denominator (ones^T E) + reciprocal + rank-1 broadcast right after
    exp so only P@V trails the V DMA
  - P@V accumulated over 32 chunks -> outT psum [d, 32]; normalize into
    attnT fp16
  - o-proj transposed layout: out[hid-chunk(p), 8 tokens-col grp] via
    lhsT = o_w 128x128 chunks, rhs = attnT -> 128 matmuls of 64 rows
    (fp16 streams 1 row/cycle at any width); stores are 2KB-contiguous
    in a [128, 32, 64] DRAM layout the host unswizzles
  - o_w streams on the otherwise-idle Pool queue; the last quarter is
    held until batch 7's V so tail compute overlaps the final transfer
"""

import sys

for _p in ("/opt/trn_rl_repo", "/root/.axon_site/_ro/trn_rl_repo"):
    if _p not in sys.path:
        sys.path.append(_p)

import numpy as np

B, Q, HID = 8, 8, 4096
H, KVH, D = 32, 8, 128
KV = 4096
ROPE_THETA = 10000.0
NCORES = 8
G = H // KVH            # 4 q-heads per kv-head (= per core)
FEAT = G * D            # 512 feature slice per core
T = B * Q               # 64 tokens
NCHUNK = KV // 128      # 32 kv chunks
HALF = D // 2
GQ = G * Q              # 32 score columns per batch

_CACHE = {}


def _build_program():
    import concourse.bass as bass
    import concourse.tile as tile
    from concourse import bacc, mybir
    from concourse.masks import make_identity
    from concourse.tile_rust import add_dep_helper
    from contextlib import ExitStack

    f32 = mybir.dt.float32
    f16 = mybir.dt.float16
    nc = bacc.Bacc("TRN2", target_bir_lowering=False, debug=False)

    xT_d = nc.dram_tensor("xt", [128, HID // 128, T], f16, kind="ExternalInput")
    qwT_d = nc.dram_tensor("qwt", [HID, FEAT], f16, kind="ExternalInput")
    owT_d = nc.dram_tensor("owt", [FEAT, HID], f16, kind="ExternalInput")
    kT_d = nc.dram_tensor("kt", [B, D, KV], f16, kind="ExternalInput")
    # v pre-swizzled on host: [B, 128(p), 32(chunk), 128(d)]
    v_d = nc.dram_tensor("v", [B, 128, NCHUNK, D], f16, kind="ExternalInput")
    # mask bias for the last kv chunk only (causal tail): [128(p), B, 32(g*q)]
    mb_d = nc.dram_tensor("mb", [128, B, GQ], f32, kind="ExternalInput")
    ones_d = nc.dram_tensor("ones", [128, 1], f16, kind="ExternalInput")
    cosb_d = nc.dram_tensor("cosb", [T, FEAT], f32, kind="ExternalInput")
    sinb_d = nc.dram_tensor("sinb", [T, FEAT], f32, kind="ExternalInput")
    # output transposed+swizzled [128(p), 32(hid chunk), 64(t)] fp16 (the
    # per-core partial is fp16; the host sums in fp32); host unswizzles
    # hid = c*128 + p
    out_d = nc.dram_tensor("out", [128, HID // 128, T], f16, kind="ExternalOutput")

    with tile.TileContext(nc) as tc, ExitStack() as ctx:
        const = ctx.enter_context(tc.tile_pool(name="const", bufs=1))
        qw_pool = ctx.enter_context(tc.tile_pool(name="qw", bufs=4))
        kt_pool = ctx.enter_context(tc.tile_pool(name="kt", bufs=2))
        v_pool = ctx.enter_context(tc.tile_pool(name="v", bufs=2))
        e_pool = ctx.enter_context(tc.tile_pool(name="e", bufs=2))
        small = ctx.enter_context(tc.tile_pool(name="small", bufs=4))
        rope_pool = ctx.enter_context(tc.tile_pool(name="rope", bufs=1))
        out_pool = ctx.enter_context(tc.tile_pool(name="outp", bufs=4))
        ps_s = ctx.enter_context(tc.tile_pool(name="ps_s", bufs=2, space="PSUM"))
        ps_o = ctx.enter_context(tc.tile_pool(name="ps_o", bufs=2, space="PSUM"))
        ps_d = ctx.enter_context(tc.tile_pool(name="ps_d", bufs=1, space="PSUM"))
        ps_b = ctx.enter_context(tc.tile_pool(name="ps_b", bufs=2, space="PSUM"))

        Exp = mybir.ActivationFunctionType.Exp
        Copy = mybir.ActivationFunctionType.Copy

        # x^T staged as [128, 32(chunk), 64] (host-swizzled, contiguous).
        # Issued first so the small DMAs' descriptor generation hides
        # under its transfer.
        xt = const.tile([128, HID // 128, T], f16)
        nc.sync.dma_start(out=xt, in_=xT_d.ap())
        ones_kv = const.tile([128, 1], f16)
        nc.sync.dma_start(out=ones_kv, in_=ones_d.ap())
        ones_bc = const.tile([1, 128], f32)
        nc.vector.memset(ones_bc, 1.0)
        ident = const.tile([T, T], f32)
        make_identity(nc, ident)
        cosb = const.tile([T, FEAT], f32)
        nc.sync.dma_start(out=cosb, in_=cosb_d.ap())
        sinb = const.tile([T, FEAT], f32)
        nc.sync.dma_start(out=sinb, in_=sinb_d.ap())
        mb31 = const.tile([128, B, GQ], f32)
        nc.sync.dma_start(out=mb31, in_=mb_d.ap())

        # ---- q projection: psum [64, 512] accumulated over 32 k-chunks
        q_ps = ps_b.tile([T, FEAT], f32, tag="misc")
        nkc = HID // 128
        QCH = 4
        qw_dmas = []
        for cgrp in range(nkc // QCH):
            qw_t = qw_pool.tile([128, QCH, FEAT], f16)
            qw_dmas.append(nc.gpsimd.dma_start(
                out=qw_t,
                in_=qwT_d.ap()
                .rearrange("(c p) f -> p c f", p=128)[
                    :, QCH * cgrp : QCH * (cgrp + 1), :
                ],
            ))
            for i in range(QCH):
                c = QCH * cgrp + i
                nc.tensor.matmul(
                    q_ps, xt[:, c, :], qw_t[:, i, :],
                    start=(c == 0), stop=(c == nkc - 1),
                )

        # ---- RoPE on the free axis (feat = g*128 + d); 1/sqrt(D) folded
        # into the host cos/sin tables
        qv = q_ps.rearrange("t (g h d) -> t g h d", g=G, h=2)
        rot = rope_pool.tile([T, G, 2, HALF], f32)
        nc.vector.tensor_copy(rot[:, :, 0, :], qv[:, :, 1, :])
        nc.vector.tensor_copy(rot[:, :, 1, :], qv[:, :, 0, :])
        q_rope = rope_pool.tile([T, FEAT], f32)
        nc.vector.tensor_mul(q_rope, q_ps, cosb)
        rot_f = rot.rearrange("t g h d -> t (g h d)")
        nc.vector.tensor_mul(rot_f, rot_f, sinb)
        nc.vector.tensor_add(q_rope, q_rope, rot_f)

        # ---- transpose each head -> qT [128(d), G, 64(b,q)] fp16
        qT = const.tile([128, G, T], f16)
        for g in range(G):
            tp = ps_b.tile([128, T], f32, tag="misc")
            nc.tensor.transpose(tp, q_rope[:, g * 128 : (g + 1) * 128], ident)
            nc.vector.tensor_copy(qT[:, g, :], tp)

        # attention output (transposed, normalized) [128(d), G, 64(b,q)]
        attnT = const.tile([128, G, T], f16)

        ow_t = const.tile([128, G, HID], f16)
        ow_dmas = {}

        def issue_ow(qi, pace_dma):
            # o_w streams on the (otherwise idle after q_w) Pool queue;
            # quarters 0-2 pace into the k/v stream, quarter 3 is held
            # until batch 7's V so tail compute overlaps its transfer
            owq = HID // 4
            dma = nc.gpsimd.dma_start(
                out=ow_t[:, :, qi * owq : (qi + 1) * owq],
                in_=owT_d.ap().rearrange("(g p) n -> p g n", p=128)[
                    :, :, qi * owq : (qi + 1) * owq
                ],
            )
            add_dep_helper(
                dma.ins, pace_dma.ins, sync=True,
                reason="pace ow quarter into the k/v stream",
            )
            ow_dmas[qi] = dma

        # ---- per-batch attention
        for b in range(B):
            kt_t = kt_pool.tile([128, KV], f16)
            kt_dma0 = nc.sync.dma_start(
                out=kt_t[:, : KV // 2], in_=kT_d.ap()[b][:, : KV // 2]
            )
            kt_dma1 = nc.sync.dma_start(
                out=kt_t[:, KV // 2 :], in_=kT_d.ap()[b][:, KV // 2 :]
            )
            v_t = v_pool.tile([128, NCHUNK, D], f16)
            v_dmas = []
            nvd = 4 if b == B - 1 else 1
            vch = NCHUNK // nvd
            for vi in range(nvd):
                v_dmas.append(nc.sync.dma_start(
                    out=v_t[:, vi * vch : (vi + 1) * vch, :],
                    in_=v_d.ap()[b][:, vi * vch : (vi + 1) * vch, :],
                ))
            if b == 0:
                # keep the q-proj weight stream ahead of batch prefetch;
                # gate on the 3rd-last qw DMA so this DMA's descriptor
                # generation overlaps the last qw transfers
                for d_inst in (kt_dma0, kt_dma1, *v_dmas):
                    add_dep_helper(
                        d_inst.ins,
                        qw_dmas[-3].ins,
                        sync=True,
                        reason="batch prefetch after q-proj weights",
                    )
            if b == 1:
                issue_ow(0, kt_dma1)
            elif b == 3:
                issue_ow(1, kt_dma1)
            elif b == 5:
                issue_ow(2, kt_dma1)
            elif b == B - 1:
                # pace on batch 7's second kt half: the ~2.2us SWDGE
                # generation + sem latency hides under the 4 v quarters,
                # so the transfer starts right as the last v quarter ends
                issue_ow(3, kt_dma1)

            # scores^T per 16-chunk group; exp is one ACT op per group
            e_t = e_pool.tile([128, NCHUNK, GQ], f16)
            for cg in range(2):
                s_ps = ps_s.tile([128, 16 * GQ], f32)
                for cc in range(16):
                    c = cg * 16 + cc
                    nc.tensor.matmul(
                        s_ps[:, cc * GQ : (cc + 1) * GQ],
                        kt_t[:, c * 128 : (c + 1) * 128],
                        qT[:, :, b * Q : (b + 1) * Q],
                        start=True,
                        stop=True,
                    )
                if cg == 1:
                    # causal mask only affects the last kv chunk
                    nc.vector.tensor_add(
                        s_ps[:, 15 * GQ :], s_ps[:, 15 * GQ :], mb31[:, b, :]
                    )
                nc.scalar.activation(
                    e_t[:, cg * 16 : (cg + 1) * 16, :].rearrange(
                        "p c j -> p (c j)"
                    ),
                    s_ps,
                    Exp,
                )

            # denominator right after exp (depends only on K): ones^T @ E,
            # chunk-halves folded in psum, then reduce + reciprocal + bcast
            d_ps = ps_d.tile([1, 16 * GQ], f32)
            nc.tensor.matmul(
                d_ps,
                ones_kv,
                e_t[:, 0:16, :].rearrange("p c j -> p (c j)"),
                start=True,
                stop=False,
            )
            nc.tensor.matmul(
                d_ps,
                ones_kv,
                e_t[:, 16:32, :].rearrange("p c j -> p (c j)"),
                start=False,
                stop=True,
            )
            den = small.tile([1, GQ], f32)
            nc.vector.reduce_sum(
                den,
                d_ps.rearrange("p (c j) -> p j c", c=16),
                axis=mybir.AxisListType.X,
            )
            rec = small.tile([1, GQ], f32)
            nc.vector.reciprocal(rec, den)
            bc_ps = ps_d.tile([128, GQ], f32, tag="bc")
            nc.tensor.matmul(bc_ps, ones_bc, rec, start=True, stop=True)
            # stage the normalizer in SBUF while PV runs, so the post-PV
            # chain is a single DVE multiply
            bc_sb = small.tile([128, GQ], f32)
            nc.scalar.activation(bc_sb, bc_ps, Copy)

            # P @ V -> outT psum [d=128, 32]
            o_ps = ps_o.tile([128, GQ], f32, tag="o")
            for c in range(NCHUNK):
                nc.tensor.matmul(
                    o_ps,
                    v_t[:, c, :],
                    e_t[:, c, :],
                    start=(c == 0),
                    stop=(c == NCHUNK - 1),
                )

            nc.vector.tensor_mul(
                attnT[:, :, b * Q : (b + 1) * Q],
                o_ps.rearrange("p (g q) -> p g q", g=G),
                bc_sb.rearrange("p (g q) -> p g q", g=G),
            )

        # ---- o-proj, transposed: outT[hid(p), t] via o_w 128x128 chunks
        # stationary, attnT moving -> 64-row fp16 matmuls (1 row/cycle).
        # 8 fine groups so copies+stores pipeline tightly behind the
        # matmul stream in the tail.
        OHC = 4  # hid 128-chunks per group
        for hg in range(HID // (128 * OHC)):
            op_ps = ps_b.tile([128, OHC, T], f32, tag="misc", name=f"op_{hg}")
            for i in range(OHC):
                hc = hg * OHC + i
                for g in range(G):
                    nc.tensor.matmul(
                        op_ps[:, i, :],
                        ow_t[:, g, hc * 128 : (hc + 1) * 128],
                        attnT[:, g, :],
                        start=(g == 0),
                        stop=(g == G - 1),
                    )
            ot = out_pool.tile([128, OHC, T], f16, tag="ot")
            # alternate copy engines so the psum->sbuf copies keep pace
            # with the matmul stream in the tail
            if hg % 2 == 0:
                nc.scalar.activation(ot, op_ps, Copy)
            else:
                nc.vector.tensor_copy(ot, op_ps)
            nc.sync.dma_start(
                out=out_d.ap()[:, hg * OHC : (hg + 1) * OHC, :],
                in_=ot,
            )

    nc.compile()
    return nc


def _get_program():
    if "nc" not in _CACHE:
        _CACHE["nc"] = _build_program()
    return _CACHE["nc"]


def _host_prep(hidden_states, position_ids, key_cache, value_cache, attention_mask, q_w, o_w):
    """Build the per-core input maps (all host-side layout marshaling)."""
    x = np.asarray(hidden_states, np.float32).reshape(T, HID).astype(np.float16)
    xT = np.ascontiguousarray(x.T.reshape(HID // 128, 128, T).transpose(1, 0, 2))

    pos = np.asarray(position_ids)
    idx = int(np.argmax(pos[0].astype(np.int32)))
    pid = pos[:, idx].astype(np.float32)                      # [B]
    inv_freq = 1.0 / (ROPE_THETA ** (np.arange(0, HALF, dtype=np.float32) / HALF))
    ang = pid[:, None] * inv_freq[None, :]                    # [B, 64]
    emb = np.concatenate([ang, ang], axis=1)                  # [B, 128]
    scale = np.float32(1.0 / np.sqrt(D))                      # folded into RoPE
    cos_b = (np.cos(emb) * scale).astype(np.float32)
    sin_b = (np.sin(emb) * scale).astype(np.float32)
    sign = np.concatenate(
        [-np.ones(HALF, np.float32), np.ones(HALF, np.float32)]
    )
    sin_s = sin_b * sign[None, :]
    cosb = np.ascontiguousarray(np.tile(np.repeat(cos_b, Q, axis=0), (1, G)))
    sinb = np.ascontiguousarray(np.tile(np.repeat(sin_s, Q, axis=0), (1, G)))

    mask = np.asarray(attention_mask)[:, 0]                   # [B, Q, KV] bool
    mbias = np.where(mask, np.float32(-10000.0), np.float32(0.0))
    # only the last kv chunk is ever masked (causal tail); ship just that
    # chunk's bias, tiled over the G q-head groups: [128(p), B, G*Q]
    mb31 = mbias[:, :, KV - 128 :].transpose(0, 2, 1)         # [B, 128, Q]
    mb_host = np.ascontiguousarray(
        np.tile(mb31, (1, 1, G)).transpose(1, 0, 2)           # [128, B, G*Q]
    )

    kc = np.asarray(key_cache, np.float32).astype(np.float16)
    vc = np.asarray(value_cache, np.float32).astype(np.float16)
    qw = np.asarray(q_w, np.float32).astype(np.float16)
    ow = np.asarray(o_w, np.float32).astype(np.float16)

    in_maps = []
    for c in range(NCORES):
        kT = np.ascontiguousarray(kc[:, c].transpose(0, 2, 1))    # [B, D, KV]
        v_sw = np.ascontiguousarray(
            vc[:, c].reshape(B, NCHUNK, 128, D).transpose(0, 2, 1, 3)
        )                                                          # [B,128,32,128]
        qwT = np.ascontiguousarray(qw[c * FEAT : (c + 1) * FEAT, :].T)  # [HID, 512]
        owT = np.ascontiguousarray(ow[:, c * FEAT : (c + 1) * FEAT].T)  # [512, HID]
        in_maps.append(
            {
                "ones": np.ones((128, 1), np.float16),
                "xt": xT,
                "qwt": qwT,
                "owt": owT,
                "kt": kT,
                "v": v_sw,
                "mb": mb_host,
                "cosb": cosb,
                "sinb": sinb,
            }
        )
    return in_maps


def kernel(
    hidden_states,
    position_ids,
    key_cache,
    value_cache,
    attention_mask,
    q_w,
    o_w,
    _trace=False,
):
    from concourse.bass_utils import run_bass_kernel_spmd

    nc = _get_program()
    in_maps = _host_prep(
        hidden_states, position_ids, key_cache, value_cache, attention_mask, q_w, o_w
    )
    res = run_bass_kernel_spmd(nc, in_maps, list(range(NCORES)), trace=_trace)
    _CACHE["last_result"] = res
    out = np.zeros((T, HID), np.float32)
    for r in res.results:
        # r["out"] is fp16 [128(p), 32(c), 64(t)] with hid = c*128 + p
        out += r["out"].astype(np.float32).transpose(1, 0, 2).reshape(HID, T).T
    return out.reshape(B, Q, HID)
